# revision 64
# baseline (speedup 1.0000x reference)
"""Bass/Trainium2 kernel for a 2-layer GCN (PyG GCNConv x2 with relu between).

Math (reference):
    A~ = A + I (self loops), deg = in-degree of A~, dis = deg^-0.5
    layer(x, W, b) = dis * (A~^T @ (dis * x) @ W) + b   (aggregation over incoming edges)
    out = layer2(relu(layer1(x, W1, b1)), W2, b2)

The symmetric normalization is folded into per-node row scalings (dis), so
edge aggregation is a pure unweighted gather + segment-sum, and the dense
64x64 matmul is applied after aggregation (associativity: A~(xW) = (A~x)W).

v2 layout (group-major, sliding windows):
  Nodes are dealt to 8 cores (degree-sorted round-robin), then within each
  core greedily packed into NGRP groups of GRP_SLOTS slots with balanced
  token counts, and sequenced inside each group so the running token count
  tracks a straight line (bounds the cross-core slot span of each window).
  Tokens (edges by target + self loops) are ordered (group, chunk,
  slot-sorted) and chopped into 128-token windows. Each window w has a
  compile-time slot offset o_w; its one-hot mask is [128 tokens x MASK_W
  slots], built by DVE is_equal of per-token segrel vs an iota row.
  The window matmul accumulates (start=False into a pre-zeroed PSUM group
  tile [128, GRP_SLOTS/2] holding [feat x slots] in two 64-partition
  halves) across ALL 4 table chunks; sub-matmuls split at PSUM 512-col bank
  boundaries. After a group's 4 chunk-cells complete, the epilogue runs
  per group: one ACT copy PSUM->SBUF(bf16), then per 128-slot block a
  64x64 weight matmul, dis scaling, bias, (relu), staged and DMA'd out.
  Two launches (one per layer); the host re-packs the bf16 table between
  layers.

Precision: "bf16" (default) gathers 256B rows with bf16 hi features (lo
half unused); "split" adds a second matmul on the bf16 lo residual for
near-fp32 accuracy.
"""

import os
import numpy as np
import concourse.bass as bass
import concourse.bacc as bacc
import concourse.mybir as mybir
from concourse.tile import TileContext
from concourse.bass_utils import run_bass_kernel_spmd

F32 = mybir.dt.float32
BF16 = mybir.dt.bfloat16
I16 = mybir.dt.int16
MODE = os.environ.get("GCN_MODE", "bf16")
USE_BF16 = True
USE_SPLIT = MODE == "split"


class Cfg:
    def __init__(self, n_nodes, cores=8):
        self.N = n_nodes
        self.CORES = cores
        self.D = 64
        self.NPC = n_nodes // cores            # real nodes per core
        self.NBLK = (self.NPC + 127) // 128    # 128-slot blocks per core
        self.NPAD = self.NBLK * 128            # padded slots per core
        self.GRP_SLOTS = 2048                  # slots per psum group
        self.NGRP = (self.NPAD + self.GRP_SLOTS - 1) // self.GRP_SLOTS
        self.TROWS = self.NPAD * cores         # table rows
        self.CHUNKR = self.TROWS // 4          # rows per gather chunk
        assert self.CHUNKR <= 32768
        self.CALL_T = 10240                    # max tokens per dma_gather call
        self.MASKW = 16                        # windows per mask tile


FULL = Cfg(100000)


# ---------------------------------------------------------------- host prep
def _chunk_balanced_order(hists):
    """Order nodes so every chunk's prefix token sum tracks a straight line
    (keeps per-(group,chunk) window boundaries aligned across cores, which
    bounds each shared window's cross-core slot span).

    hists: [n, 4] per-node chunk token histograms. Greedy: at each slot,
    consider the largest/smallest remaining nodes (by total) and take the
    one minimizing the summed squared per-chunk deviation from the line.
    """
    n = len(hists)
    if n == 0:
        return np.empty(0, np.int64)
    tot = hists.sum(axis=1)
    order = list(np.argsort(-tot, kind="stable"))
    rates = hists.sum(axis=0) / n  # per-slot target per chunk
    cum = np.zeros(4)
    seq = np.empty(n, np.int64)
    K = 8
    for j in range(n):
        m = len(order)
        pos = list(range(min(K, m))) + list(range(max(K, m - K), m))
        target = (j + 1) * rates
        best, bestv = None, None
        for p in pos:
            idx = order[p]
            d = cum + hists[idx] - target
            v = float(d @ d)
            if bestv is None or v < bestv:
                best, bestv = p, v
        pick = order[best]
        seq[j] = pick
        cum += hists[pick]
        del order[best]
    return seq


def _prepare(cfg, edge_index):
    """Build per-core token streams and the shared SPMD schedule."""
    src = np.asarray(edge_index[0], dtype=np.int64)
    tgt = np.asarray(edge_index[1], dtype=np.int64)
    N, C, G = cfg.N, cfg.CORES, cfg.GRP_SLOTS

    deg = np.bincount(tgt, minlength=N).astype(np.int64) + 1  # tokens per node
    dis = (deg.astype(np.float32)) ** np.float32(-0.5)

    # degree-sorted round-robin deal: rank i -> core i%C (equalizes per-core
    # token totals and degree profiles)
    order = np.argsort(deg, kind="stable")
    node_core = np.empty(N, np.int32)
    node_core[order] = (np.arange(N) % C).astype(np.int32)

    # table chunk per node: fixed per-core round-robin by rank, DECOUPLED
    # from the slot assignment so all token chunks are known before slot
    # sequencing
    qtr = cfg.NPAD // 4
    node_chunk = np.empty(N, np.int64)
    node_rank4 = np.empty(N, np.int64)  # rank // 4 within (core, chunk)
    for q in range(C):
        own = np.flatnonzero(node_core == q)
        node_chunk[own] = np.arange(len(own)) % 4
        node_rank4[own] = np.arange(len(own)) // 4
    trow = node_chunk * cfg.CHUNKR + node_core.astype(np.int64) * qtr + node_rank4

    # per-node chunk token histograms (in-edge sources' chunks only; self
    # loops are handled by a streamed per-slot table + identity matmul,
    # not gather tokens)
    hists = np.zeros((N, 4), np.int64)
    np.add.at(hists, (tgt, node_chunk[src]), 1)

    # per-core: greedy PER-CHUNK token-balanced deal into groups. Balancing
    # each chunk's count (not just the total) aligns every (group, chunk)
    # cell's size across cores, which is what bounds the shared windows'
    # cross-core slot drift. Then chunk-balanced sequencing within each
    # group -> node_loc.
    caps = [min(G, cfg.NPAD - g * G) for g in range(cfg.NGRP)]
    capsf = np.array(caps, np.float64)
    node_loc = np.empty(N, np.int32)
    for q in range(C):
        own = np.flatnonzero(node_core == q)
        o2 = own[np.argsort(-deg[own], kind="stable")]
        gtok = np.zeros(cfg.NGRP)
        gcnt = np.zeros(cfg.NGRP, np.int64)
        assign = np.empty(len(o2), np.int32)
        for i, n_ in enumerate(o2):
            best, bestv = -1, None
            for g in range(cfg.NGRP):
                if gcnt[g] >= caps[g]:
                    continue
                v = (gtok[g] + deg[n_]) / capsf[g]
                if bestv is None or v < bestv:
                    best, bestv = g, v
            assign[i] = best
            gtok[best] += deg[n_]
            gcnt[best] += 1
        for g in range(cfg.NGRP):
            nodes_g = o2[assign == g]
            seq = _chunk_balanced_order(hists[nodes_g].astype(np.float64))
            node_loc[nodes_g[seq]] = g * G + np.arange(len(nodes_g))

    # token lists (edges by target core; self loops are not tokens)
    all_srcrow = trow[src]
    all_tcore = node_core[tgt]
    all_tloc = node_loc[tgt]
    chunk = (all_srcrow // cfg.CHUNKR).astype(np.int32)
    grp = (all_tloc // G).astype(np.int32)

    # per-(core, grp, chunk) counts -> shared window counts
    counts = np.zeros((C, cfg.NGRP, 4), np.int64)
    np.add.at(counts, (all_tcore, grp, chunk), 1)
    n_win = np.maximum(1, (counts.max(axis=0) + 127) // 128)  # [NGRP, 4]
    W_total = int(n_win.sum())
    T_total = W_total * 128
    total_real = int(counts.sum())
    pad_frac = (T_total * C - total_real) / max(total_real, 1)

    # per-core sorted token arrays per cell, and window slot extents
    win_start = np.zeros((cfg.NGRP, 4), np.int64)  # window index of cell start
    acc = 0
    for g in range(cfg.NGRP):
        for c in range(4):
            win_start[g, c] = acc
            acc += int(n_win[g, c])

    per_core_tok = []  # [core][(g,c)] -> (srcrow_sorted, srel_sorted)
    wmin = np.full(W_total, 1 << 30, np.int64)
    wmax = np.full(W_total, -1, np.int64)
    for q in range(C):
        m = all_tcore == q
        csrc, cgrp, cchunk, ctloc = (
            all_srcrow[m], grp[m], chunk[m], all_tloc[m])
        srel = ctloc - cgrp.astype(np.int64) * G
        so = np.lexsort((srel, cchunk, cgrp))
        csrc, cgrp, cchunk, srel = csrc[so], cgrp[so], cchunk[so], srel[so]
        keys = cgrp.astype(np.int64) * 4 + cchunk
        cells = {}
        starts = np.concatenate([[0], np.flatnonzero(np.diff(keys)) + 1])
        ends = np.concatenate([starts[1:], [len(keys)]])
        for s, e in zip(starts, ends):
            g, c = int(cgrp[s]), int(cchunk[s])
            nt = e - s
            # spread this core's pad slots uniformly through the cell so
            # every core's real tokens track the same fractional position
            # -> slot line (kills cross-core window drift at no cost)
            Tc = int(n_win[g, c]) * 128
            pos = (np.arange(nt, dtype=np.int64) * Tc) // nt
            cells[(g, c)] = (csrc[s:e], srel[s:e], pos)
            # window extents over placed positions
            w0 = int(win_start[g, c])
            wloc = pos // 128
            np.minimum.at(wmin, w0 + wloc, srel[s:e])
            np.maximum.at(wmax, w0 + wloc, srel[s:e])
        per_core_tok.append(cells)

    # per-window offsets and width class (shared): pick the narrow mask
    # width from the span distribution (smaller width = cheaper DVE mask
    # build + cheaper PE), spilling oversized windows to 128-wide masks
    span = int((wmax - np.minimum(wmin, wmax)).max()) + 1
    assert span <= 128, f"window slot span {span} exceeds 128"
    spans = (wmax - np.minimum(wmin, wmax) + 1).astype(np.int64)
    best_w, best_cost = 64, None
    for wcand in (32, 48, 64, 96):
        nnar = int((spans <= wcand).sum())
        nwid = len(spans) - nnar
        cost = nnar * (wcand * 0.2604 + 1.9 + wcand * 0.4167) + \
            nwid * (128 * 0.5208 + 3.75 + 128 * 0.4167)
        if best_cost is None or cost < best_cost:
            best_w, best_cost = wcand, cost
    NARROW_W = best_w
    windows = []  # (g, c, o_w, wide, cls_idx)
    n_narrow = n_wide = 0
    for g in range(cfg.NGRP):
        for c in range(4):
            for wl in range(int(n_win[g, c])):
                wi = int(win_start[g, c]) + wl
                mn = int(wmin[wi]) if wmax[wi] >= 0 else 0
                mx = int(wmax[wi]) if wmax[wi] >= 0 else 0
                o = min(mn, max(caps[g] - NARROW_W, 0))
                wide = mx - o >= NARROW_W
                if wide:
                    o = min(mn, max(caps[g] - 128, 0))
                    windows.append((g, c, o, True, n_wide))
                    n_wide += 1
                else:
                    windows.append((g, c, o, False, n_narrow))
                    n_narrow += 1

    # gather calls: contiguous token ranges within one cell, <= CALL_T.
    # The first cell is split into escalating pieces so the window pipeline
    # starts ~2us into the kernel; the last cell into shrinking pieces so
    # the tail windows aren't stuck behind one big transfer.
    calls = []  # (chunk, tok_start, ntok)
    cell_list = [(g, c) for g in range(cfg.NGRP) for c in range(4)]
    for ci_, (g, c) in enumerate(cell_list):
        t0 = int(win_start[g, c]) * 128
        t1 = t0 + int(n_win[g, c]) * 128
        t = t0
        while t < t1:
            nt = min(cfg.CALL_T, t1 - t)
            if ci_ == 0 and t - t0 < 3072:
                nt = min(1024, t1 - t)
            calls.append((c, t, nt))
            t += nt

    # per-core idx/segrel streams (segrel split by window width class)
    o_of_w = np.array([w[2] for w in windows], np.int64)
    mw_of_w = np.where(np.array([w[3] for w in windows]), 128, NARROW_W)
    per_core = []
    for q in range(C):
        idx16 = np.zeros(T_total, np.int16)
        segrel = np.full(T_total, -1, np.int16)
        for g in range(cfg.NGRP):
            for c in range(4):
                if (g, c) not in per_core_tok[q]:
                    continue
                csrc, srel, pos = per_core_tok[q][(g, c)]
                base = int(win_start[g, c]) * 128
                wis = int(win_start[g, c]) + pos // 128
                idx16[base + pos] = (csrc - c * cfg.CHUNKR).astype(np.int16)
                sr = srel - o_of_w[wis]
                assert (sr >= 0).all() and (sr < mw_of_w[wis]).all()
                segrel[base + pos] = sr.astype(np.int16)
        idxw = np.tile(idx16.reshape(T_total // 16, 16).T, (8, 1)).copy()
        seg2 = segrel.reshape(W_total, 128)
        segn = np.full((max(n_narrow, 1), 128), -1, np.int16)
        segwd = np.full((max(n_wide, 1), 128), -1, np.int16)
        for wi, (g, c, o, wide, ci) in enumerate(windows):
            (segwd if wide else segn)[ci] = seg2[wi]
        disq = np.ones(cfg.NPAD, np.float32)
        own = np.flatnonzero(node_core == q)
        disq[node_loc[own]] = dis[own]
        disb = disq.reshape(cfg.NBLK, 128).T.copy()
        per_core.append(dict(idxw=idxw, segn=segn.T.copy(),
                             segwd=segwd.T.copy(), disb=disb, disq=disq))

    meta = dict(
        windows=windows,
        calls=calls,
        n_win=n_win,
        win_start=win_start,
        W_total=W_total,
        T_total=T_total,
        n_narrow=n_narrow,
        n_wide=n_wide,
        NARROW_W=NARROW_W,
        pad_frac=pad_frac,
        span=span,
        node_core=node_core,
        node_loc=node_loc,
        trow=trow,
        dis=dis,
        per_core=per_core,
    )
    return meta


# ------------------------------------------------------------- kernel build
def _build_layer_nc(cfg, meta, relu):
    """One GCN layer as a Tile kernel. relu=True for layer 1."""
    nc = bacc.Bacc(None, target_bir_lowering=False)
    T, Wn = meta["T_total"], meta["W_total"]
    D, NBLK, NGRP, G = cfg.D, cfg.NBLK, cfg.NGRP, cfg.GRP_SLOTS
    NN = max(meta["n_narrow"], 1)
    NW = max(meta["n_wide"], 1)
    NARW = meta["NARROW_W"]
    MASKW_N = 32  # narrow windows per mask tile
    MASKW_W = 16  # wide windows per mask tile
    GW = G // 2  # psum group tile cols (two 64-partition halves)

    E2 = bool(os.environ.get("GCN_ELEM512"))  # timing probe: 512B descriptors
    ED = 2 * D if E2 else D
    SPKT = bool(os.environ.get("GCN_SINGLE_PACKET"))
    table = nc.declare_dram_parameter("table", [cfg.TROWS, ED], F32, isOutput=False)
    idxw_d = nc.declare_dram_parameter("idxw", [128, T // 16], I16, isOutput=False)
    segn_d = nc.declare_dram_parameter("segn", [128, NN], I16, isOutput=False)
    segwd_d = nc.declare_dram_parameter("segwd", [128, NW], I16, isOutput=False)
    disb_d = nc.declare_dram_parameter("disb", [128, NBLK], F32, isOutput=False)
    bt_d = nc.declare_dram_parameter("bt", [128, D], F32, isOutput=False)
    w_d = nc.declare_dram_parameter("w", [D, D], F32, isOutput=False)
    selft_d = nc.declare_dram_parameter("selft", [cfg.NPAD, D], BF16, isOutput=False)
    hout = nc.declare_dram_parameter("hout", [cfg.NPAD, D], F32, isOutput=True)

    windows = meta["windows"]
    calls = meta["calls"]
    n_win = meta["n_win"]
    win_start = meta["win_start"]
    caps = [min(G, cfg.NPAD - g * G) for g in range(NGRP)]
    hout_r = hout[:].rearrange("(b p) d -> p b d", p=128)

    with TileContext(nc) as tc:
        with (
            tc.tile_pool(name="const", bufs=1) as cpool,
            tc.tile_pool(name="msg", bufs=3) as mpool,
            tc.tile_pool(name="mask", bufs=3) as kpool,
            tc.tile_pool(name="zsb", bufs=2) as zpool,
            tc.tile_pool(name="sv", bufs=2) as svpool,
            tc.tile_pool(name="stage", bufs=2) as spool,
            tc.tile_pool(name="eptmp", bufs=3) as epool,
            tc.tile_pool(name="grp", bufs=3, space="PSUM") as gpool,
            tc.tile_pool(name="p2", bufs=2, space="PSUM") as p2pool,
        ):
            # DMA order tuned for the startup critical path: the first
            # descgen needs only the first index piece; the first mask
            # build needs segn + the iota table.
            wtf = cpool.tile([128, D], F32)
            nc.sync.dma_start(out=wtf[0:D, :], in_=w_d[:])
            nc.sync.dma_start(out=wtf[D:2 * D, :], in_=w_d[:])
            idxw = cpool.tile([128, T // 16], I16)
            ncol = T // 16
            bounds = [0, min(256, ncol)]
            step = (ncol + NGRP - 1) // NGRP
            while bounds[-1] < ncol:
                bounds.append(min(ncol, bounds[-1] + step))
            nc.sync.dma_start(
                out=idxw[:, 0:bounds[1]], in_=idxw_d[:, 0:bounds[1]])
            segn = cpool.tile([128, NN], I16)
            nc.sync.dma_start(out=segn[:], in_=segn_d[:])
            iota_exp = cpool.tile([128, 128, MASKW_N], I16)
            nc.gpsimd.iota(
                iota_exp[:], pattern=[[1, 128], [0, MASKW_N]], base=0,
                channel_multiplier=0,
            )
            # identity mask for the streamed self-loop adds: ident[p, c] =
            # (p == c), multiplying sv [slot, feat] into Z's [feat, slot]
            iota_p = cpool.tile([128, 128], I16)
            nc.gpsimd.iota(iota_p[:], pattern=[[0, 128]], base=0,
                           channel_multiplier=1)
            ident = cpool.tile([128, 128], BF16)
            nc.vector.tensor_tensor(
                out=ident[:], in0=iota_p[:], in1=iota_exp[:, :, 0],
                op=mybir.AluOpType.is_equal,
            )
            segwd = cpool.tile([128, NW], I16)
            nc.sync.dma_start(out=segwd[:], in_=segwd_d[:])
            disb = cpool.tile([128, NBLK], F32)
            nc.sync.dma_start(out=disb[:], in_=disb_d[:])
            bt = cpool.tile([128, D], F32)
            nc.sync.dma_start(out=bt[:], in_=bt_d[:])
            wbt = cpool.tile([128, D], BF16)
            nc.vector.tensor_copy(out=wbt[:], in_=wtf[:])
            zeros = cpool.tile([128, GW], F32)
            nc.gpsimd.memset(zeros[:], 0.0)
            for c0, c1 in zip(bounds[1:-1], bounds[2:]):
                nc.sync.dma_start(out=idxw[:, c0:c1], in_=idxw_d[:, c0:c1])

            call_i = 0
            msg_tile = None
            msg_base = 0
            mask_n = mask_w = None
            nbase = wbase = -1

            selft_r = selft_d[:].rearrange("(b p) d -> p b d", p=128)
            for g in range(NGRP):
                grp_tile = gpool.tile([128, GW], F32, tag="grp")
                nc.scalar.activation(
                    out=grp_tile[:], in_=zeros[:],
                    func=mybir.ActivationFunctionType.Copy,
                )
                # streamed self-loop rows for this group's slots (issued on
                # the ACT engine's DMA queue so it isn't stuck behind the
                # index-stream loads); consumed after the cell windows
                nbg = (caps[g] + 127) // 128
                hbg = GW // 128
                sv = svpool.tile([128, G // 128, D], BF16, tag="sv")
                gb = g * (G // 128)
                nc.scalar.dma_start(
                    out=sv[:, :nbg, :], in_=selft_r[:, gb:gb + nbg, :])
                # accumulate the 4 chunk-cells of this group
                last_wi = int(win_start[g, 3]) + int(n_win[g, 3]) - 1
                for c in range(4):
                    w0 = int(win_start[g, c])
                    for wl in range(int(n_win[g, c])):
                        wi = w0 + wl
                        tok = wi * 128
                        if call_i < len(calls) and calls[call_i][1] == tok:
                            cc, t0, ntok = calls[call_i]
                            nslots = ntok // 128
                            msg_tile = mpool.tile(
                                [128, cfg.CALL_T // 128, ED], F32, tag="msg")
                            if os.environ.get("SKIP_GATHER"):
                                nc.any.memset(msg_tile[:, :nslots, :], 0.0)
                            else:
                                nc.gpsimd.dma_gather(
                                    msg_tile[:, :nslots, :],
                                    table[cc * cfg.CHUNKR:(cc + 1) * cfg.CHUNKR, :],
                                    idxw[:, t0 // 16:(t0 + ntok) // 16],
                                    num_idxs=ntok,
                                    num_idxs_reg=ntok,
                                    elem_size=ED,
                                    single_packet=SPKT,
                                )
                            msg_base = t0
                            call_i += 1
                        _, _, o, wide, ci = windows[wi]
                        if wide:
                            if wbase < 0 or ci - wbase >= MASKW_W:
                                nw = min(MASKW_W, NW - ci)
                                mask_w = kpool.tile(
                                    [128, 128, MASKW_W], BF16, tag="maskw")
                                nc.vector.tensor_tensor(
                                    out=mask_w[:, :, :nw],
                                    in0=segwd[:, ci:ci + nw]
                                    .rearrange("p (o w) -> p o w", o=1)
                                    .to_broadcast([128, 128, nw]),
                                    in1=iota_exp[:, :, :nw],
                                    op=mybir.AluOpType.is_equal,
                                )
                                wbase = ci
                            mtile, mbase, width = mask_w, wbase, 128
                        else:
                            if nbase < 0 or ci - nbase >= MASKW_N:
                                nw = min(MASKW_N, NN - ci)
                                mask_n = kpool.tile(
                                    [128, NARW, MASKW_N], BF16, tag="maskn")
                                nc.vector.tensor_tensor(
                                    out=mask_n[:, :, :nw],
                                    in0=segn[:, ci:ci + nw]
                                    .rearrange("p (o w) -> p o w", o=1)
                                    .to_broadcast([128, NARW, nw]),
                                    in1=iota_exp[:, 0:NARW, :nw],
                                    op=mybir.AluOpType.is_equal,
                                )
                                nbase = ci
                            mtile, mbase, width = mask_n, nbase, NARW
                        mview = msg_tile[:, (tok - msg_base) // 128, 0:D].bitcast(BF16)
                        # split at psum 512-col boundaries (and the
                        # 64-partition half boundary at GW)
                        segs = []
                        s0 = o
                        send = o + width
                        while s0 < send:
                            s1 = min(send, (s0 // 512 + 1) * 512)
                            segs.append((s0, s1))
                            s0 = s1
                        for si, (s0, s1) in enumerate(segs):
                            h = s0 // GW
                            c0 = s0 - h * GW
                            out_ap = grp_tile[64 * h:64 * h + 64, c0:c0 + (s1 - s0)]
                            rhs_ap = mtile[:, s0 - o:s1 - o, ci - mbase]
                            nc.tensor.matmul(
                                out=out_ap, lhsT=mview[:, 0:D], rhs=rhs_ap,
                                start=False, stop=False,
                                skip_group_check=True,
                            )
                            if USE_SPLIT:
                                nc.tensor.matmul(
                                    out=out_ap, lhsT=mview[:, D:2 * D],
                                    rhs=rhs_ap, start=False, stop=False,
                                    skip_group_check=True,
                                )

                # self-loop adds via identity matmuls (close the group's
                # psum accumulation: stop on the last one)
                for j in range(nbg):
                    h = j // hbg
                    cols = 128 * (j % hbg)
                    nc.tensor.matmul(
                        out=grp_tile[64 * h:64 * h + 64, cols:cols + 128],
                        lhsT=sv[:, j, :], rhs=ident[:],
                        start=False, stop=j == nbg - 1, skip_group_check=True,
                    )

                # epilogue for this group
                zsb = zpool.tile([128, GW], BF16, tag="zsb")
                nc.scalar.activation(
                    out=zsb[:], in_=grp_tile[:],
                    func=mybir.ActivationFunctionType.Copy,
                )
                nb = (caps[g] + 127) // 128
                hb = GW // 128  # blocks per 64-partition half
                stage = spool.tile([128, G // 128, D], F32, tag="stage")
                for j in range(nb):
                    B = g * (G // 128) + j
                    h = j // hb
                    cols = 128 * (j % hb)
                    ps2 = p2pool.tile([128, D], F32, tag="p2")
                    nc.tensor.matmul(
                        out=ps2[:],
                        lhsT=zsb[64 * h:64 * h + 64, cols:cols + 128],
                        rhs=wbt[64 * h:64 * h + 64, :],
                        start=True, stop=True,
                    )
                    if relu:
                        sc = epool.tile([128, D], F32, tag="sc")
                        nc.scalar.activation(
                            out=sc[:], in_=ps2[:],
                            func=mybir.ActivationFunctionType.Copy,
                            scale=disb[:, B:B + 1],
                        )
                        tmp = epool.tile([128, D], F32, tag="tmp")
                        nc.vector.tensor_tensor(
                            out=tmp[:], in0=sc[:], in1=bt[:],
                            op=mybir.AluOpType.add,
                        )
                        nc.scalar.activation(
                            out=stage[:, j, :], in_=tmp[:],
                            func=mybir.ActivationFunctionType.Relu,
                            scale=disb[:, B:B + 1],
                        )
                    else:
                        tmp = epool.tile([128, D], F32, tag="tmp")
                        nc.scalar.activation(
                            out=tmp[:], in_=ps2[:],
                            func=mybir.ActivationFunctionType.Copy,
                            scale=disb[:, B:B + 1],
                        )
                        nc.vector.tensor_tensor(
                            out=stage[:, j, :], in0=tmp[:], in1=bt[:],
                            op=mybir.AluOpType.add,
                        )
                b0 = g * (G // 128)
                nc.sync.dma_start(
                    out=hout_r[:, b0:b0 + nb, :], in_=stage[:, :nb, :]
                )

    nc.compile()
    return nc


# ---------------------------------------------------------------- execution
_CACHE = {}


def _get_built(cfg, meta):
    key = ("nc", cfg.N, meta["W_total"], meta["n_wide"], meta["NARROW_W"], USE_SPLIT)
    if key not in _CACHE:
        _CACHE[key] = (
            _build_layer_nc(cfg, meta, relu=True),
            _build_layer_nc(cfg, meta, relu=False),
        )
    return _CACHE[key]


def _pack_selft(cfg, meta, scaled_by_node):
    """Per-core [NPAD, D] bf16 self-loop tables ordered by slot: row loc =
    (dis*x)[n] for the node at (core, loc); pad slots zero. Input is the
    already dis-scaled per-node matrix [N, D]."""
    import ml_dtypes
    out = []
    nc_, nl_ = meta["node_core"], meta["node_loc"]
    for q in range(cfg.CORES):
        own = np.flatnonzero(nc_ == q)
        st = np.zeros((cfg.NPAD, cfg.D), ml_dtypes.bfloat16)
        st[nl_[own]] = scaled_by_node[own].astype(ml_dtypes.bfloat16)
        out.append(st)
    return out


def _run_layer(nc, cfg, meta, table, selfts, wmat, bvec, trace=False):
    if table.dtype != np.float32:
        table = table.view(np.float32)
    bt = np.tile(bvec[None, :], (128, 1)).astype(np.float32)
    in_maps = []
    for q in range(cfg.CORES):
        pc = meta["per_core"][q]
        in_maps.append(
            dict(
                table=table,
                idxw=pc["idxw"],
                segn=pc["segn"],
                segwd=pc["segwd"],
                disb=pc["disb"],
                selft=selfts[q],
                bt=bt,
                w=np.ascontiguousarray(wmat, dtype=np.float32),
            )
        )
    res = run_bass_kernel_spmd(
        nc, in_maps, core_ids=list(range(cfg.CORES)), trace=trace
    )
    shards = [res.results[q]["hout"] for q in range(cfg.CORES)]
    return shards, res


def _pack_table(cfg, rows, vals):
    """vals [n, D] f32 -> bf16 table rows (hi, optional lo residual)."""
    import ml_dtypes
    tb = np.zeros((cfg.TROWS, 2 * cfg.D), ml_dtypes.bfloat16)
    hi = vals.astype(ml_dtypes.bfloat16)
    tb[rows, :cfg.D] = hi
    if USE_SPLIT:
        tb[rows, cfg.D:] = (vals - hi.astype(np.float32)).astype(ml_dtypes.bfloat16)
    return tb


def gcn_forward(cfg, x, edge_index, W1, b1, W2, b2, trace=False):
    key = ("meta", cfg.N, int(np.asarray(edge_index).sum()) & 0xFFFFFFFF)
    if key not in _CACHE:
        _CACHE[key] = _prepare(cfg, edge_index)
    meta = _CACHE[key]
    nc1, nc2 = _get_built(cfg, meta)

    dis = meta["dis"]
    trow = meta["trow"]
    xp = np.asarray(x, np.float32) * dis[:, None]
    table1 = _pack_table(cfg, trow, xp)
    selfts1 = _pack_selft(cfg, meta, xp)

    shards1, res1 = _run_layer(nc1, cfg, meta, table1, selfts1, W1, b1,
                               trace=trace)

    import ml_dtypes
    nc_, nl_ = meta["node_core"], meta["node_loc"]
    # layer-1's epilogue already scaled its output by dis (H' = dis*relu(..)),
    # so both the gather table and the self table take shards1 rows as-is
    hscaled = np.empty((cfg.N, cfg.D), np.float32)
    table2 = np.zeros((cfg.TROWS, 2 * cfg.D), ml_dtypes.bfloat16)
    for q in range(cfg.CORES):
        own = np.flatnonzero(nc_ == q)
        vals = shards1[q][nl_[own]]
        hscaled[own] = vals
        hi = vals.astype(ml_dtypes.bfloat16)
        table2[trow[own], :cfg.D] = hi
        if USE_SPLIT:
            table2[trow[own], cfg.D:] = (
                vals - hi.astype(np.float32)
            ).astype(ml_dtypes.bfloat16)
    selfts2 = _pack_selft(cfg, meta, hscaled)
    shards2, res2 = _run_layer(nc2, cfg, meta, table2, selfts2, W2, b2,
                               trace=trace)

    out = np.empty((cfg.N, cfg.D), np.float32)
    allsh = np.concatenate(shards2, axis=0)
    out[:] = allsh[nc_.astype(np.int64) * cfg.NPAD + nl_]
    return out, (res1, res2)


def kernel(x, edge_index, W1, b1, W2, b2):
    out, _ = gcn_forward(
        FULL,
        np.asarray(x),
        np.asarray(edge_index),
        np.asarray(W1),
        np.asarray(b1),
        np.asarray(W2),
        np.asarray(b2),
    )
    return out
